# revision 6
# baseline (speedup 1.0000x reference)
# GQA causal attention with RoPE on 8 TRN2 NeuronCores (tensor-parallel over
# heads), restructured so every phase overlaps the ACT-bound attention windows.
#
# Schedule (emission order = engine-queue order):
#   head : projections+RoPE for batch 0 (PE-bound, ~82us)
#   S1   : attention (h0,b0)  + filler = batch-1 projections (first half)
#   S2   : attention (h1,b0)  + filler = batch-1 projections (rest)
#   S3   : attention (h0,b1)  + filler = o_proj rows 0-2 (batch-0 rows)
#   S4   : attention (h1,b1)  + filler = o_proj row 3 + batch-1 hp0 partials
#   tail : final AllToAll + o_proj batch-1 hp1 + combine with stashed partials
#
# AllToAll is per (head, batch) (4 half-size collectives, one per section
# end), which re-shards the output rows: core c owns rows [c*512,(c+1)*512)
# of batch 0 AND of batch 1; the host interleaves the 8 shards accordingly.
#
# Other changes vs the two-phase layout: RoPE runs bf16 (scalar-engine copy
# PSUM->bf16, then 2x-rate DVE muls), V reaches its natural [kv,HD] layout
# via DMA transposes (no PE transposes, no PSUM bank), the softmax-normalize
# multiply reads its two PSUM operands directly, and the denominator /
# broadcast matmuls allocate from the score-PSUM pool so projections and
# o_proj each get two PSUM banks for their interleaved accumulations.

import math
import sys

for _p in ("/opt/trn_rl_repo",):
    if _p not in sys.path:
        sys.path.insert(0, _p)

import numpy as np
import ml_dtypes

B = 2
S = 4096
D = 2048
H = 16
KVH = 4
HD = 128
N_CORES = 8
BS = B * S
SHARD = BS // N_CORES       # 1024 output rows per core
HSH = SHARD // 2            # 512 rows per (core, batch)
HPC = H // N_CORES          # 2 q heads per core
SCALE = 1.0 / math.sqrt(HD)

SQ = 512                    # q-block (matmul free dim)
KV = 128                    # kv-block (psum partition dim)
DCH = D // 128              # 16 contraction chunks for the projections
NB = S // SQ                # 8 q-blocks per batch
NKV_B = S // KV             # 32 kv-blocks per batch
DIAG = SQ // KV             # 4 kv-blocks per q-block on the causal diagonal

BF16 = ml_dtypes.bfloat16

_CACHE = {}
CUR = ['start']


class _Filler:
    """Paced emission of generator units + time-deferred thunks."""

    def __init__(self):
        self.gen = None
        self.step = 0
        self.thunks = []        # (due_step, fn)
        self.credit = 0.0
        self.rate = 0.0

    def set_gen(self, gen, rate):
        self.gen = gen
        self.rate = rate
        self.credit = 0.0

    def defer(self, delay, fn):
        self.thunks.append((self.step + delay, fn))

    def _one(self):
        self.step += 1
        if self.thunks:
            due = [t for t in self.thunks if t[0] <= self.step]
            if due:
                self.thunks = [t for t in self.thunks if t[0] > self.step]
                for _, fn in due:
                    fn()
        if self.gen is not None:
            try:
                next(self.gen)
            except StopIteration:
                self.gen = None

    def take(self):
        self.credit += self.rate
        while self.credit >= 1.0:
            self.credit -= 1.0
            self._one()

    def drain(self, thunks=True):
        while self.gen is not None or (thunks and self.thunks):
            self._one()


def _build(sim_mode=False):
    import concourse.mybir as mybir
    import concourse.tile as tile
    from concourse import bacc

    dt = mybir.dt
    nc = bacc.Bacc("TRN2", target_bir_lowering=False, debug=False,
                   enable_asserts=True, num_devices=N_CORES)

    # ---- external inputs (per-core shards supplied via in_maps) ----
    xT = nc.dram_tensor("xT", [D, BS], dt.bfloat16, kind="ExternalInput")
    cosT = nc.dram_tensor("cosT", [HD, S], dt.bfloat16, kind="ExternalInput")
    sinTs = nc.dram_tensor("sinTs", [HD, S], dt.bfloat16, kind="ExternalInput")
    wq = nc.dram_tensor("wq", [D, HPC * HD], dt.bfloat16, kind="ExternalInput")
    # kv weight: even cores carry their kv-head's K columns, odd cores the V
    # columns; the pair exchanges results (same SPMD program, different data)
    wkv = nc.dram_tensor("wkv", [D, HD], dt.bfloat16, kind="ExternalInput")
    # per-core kv-rope blend flags: 1 on K cores (apply rope), 0 on V cores
    kvf = nc.dram_tensor("kvf", [128, 1], dt.float32, kind="ExternalInput")
    kvi = nc.dram_tensor("kvi", [128, 1], dt.float32, kind="ExternalInput")
    wo = nc.dram_tensor("wo", [D, D], dt.bfloat16, kind="ExternalInput")
    masks = nc.dram_tensor("masks", [128, DIAG * SQ], dt.bfloat16,
                           kind="ExternalInput")
    onesb = nc.dram_tensor("onesb", [128, 1], dt.bfloat16, kind="ExternalInput")
    onesf = nc.dram_tensor("onesf", [1, 128], dt.float32, kind="ExternalInput")

    out = nc.dram_tensor("out", [SHARD, D], dt.bfloat16, kind="ExternalOutput")

    # ---- internal DRAM for the pairwise k/v exchange (per batch, per half) ----
    ktv_dram = [nc.dram_tensor(f"ktv{b}", [2, HD, S // 2], dt.bfloat16)
                for b in range(B)]
    kv_pair = [nc.dram_tensor(f"kvp{b}", [2, 2, HD, S // 2], dt.bfloat16)
               for b in range(B)]

    # ---- internal DRAM for the per-(head,batch) AllToAll ----
    # chunk j of ao_in holds this core's head-h ctx for q-cols [j*512,(j+1)*512)
    # of batch b; after A2A chunk j holds core j's head for THIS core's cols.
    ao_in = [[nc.dram_tensor(f"ao_in{h}_{b}", [N_CORES, HD, HSH], dt.bfloat16)
              for b in range(B)] for h in range(HPC)]
    ao_ex = [[nc.dram_tensor(f"ao_ex{h}_{b}", [N_CORES, HD, HSH], dt.bfloat16)
              for b in range(B)] for h in range(HPC)]
    if sim_mode:
        # timing-only runs alias every exchange output to its input: the
        # consumer-side dependency (can't read before the ship) is kept, the
        # network itself is not modeled (final hop covered by the reported
        # +25us tail allowance, mid-kernel hops hidden by construction)
        ao_ex = ao_in
        kv_src = [[ktv_dram[b][hx], ktv_dram[b][hx]] for b in range(B)
                  for hx in ()]  # unused placeholder


    with tile.TileContext(nc) as tc:
      with tc.tile_pool(name="persist", bufs=1) as pp, \
           tc.tile_pool(name="qkv1", bufs=1) as q1p, \
           tc.tile_pool(name="et", bufs=5) as etp, \
           tc.tile_pool(name="accp", bufs=2) as accp, \
           tc.tile_pool(name="recp", bufs=2) as recp, \
           tc.tile_pool(name="bcp", bufs=2) as bcp, \
           tc.tile_pool(name="aobp", bufs=1) as aobp, \
           tc.tile_pool(name="wop", bufs=1) as wop, \
           tc.tile_pool(name="ktvp", bufs=1) as ktvp, \
           tc.tile_pool(name="pscp", bufs=2, space="PSUM") as pscp, \
           tc.tile_pool(name="pop", bufs=2, space="PSUM") as pop:
        mask_sb = pp.tile([128, DIAG * SQ], dt.bfloat16, name="mask_sb")
        ob_sb = pp.tile([128, 1], dt.bfloat16, name="ob_sb")
        of_sb = pp.tile([1, 128], dt.float32, name="of_sb")

        qts = {1: q1p.tile([HD, HPC, S], dt.bfloat16, name="qt1")}
        kts = {1: q1p.tile([HD, S], dt.bfloat16, name="kt1")}
        vns = {1: q1p.tile([128, NKV_B, HD], dt.bfloat16, name="vn1")}
        ktvs = {1: ktvp.tile([HD, S], dt.bfloat16, name="ktv1", tag="ktv")}

        filler = _Filler()
        # wo in two half-tiles: wo_ab[0] resident from the start (loaded
        # during S1/S2 slack), wo_ab[1] allocated+loaded at S3
        wo_ab = [wop.tile([128, DCH, D // 2], dt.bfloat16, name="wo_a"), None]

        def wo_slice(g, dj, c0, c1):
            t = wo_ab[dj // 2]
            base = (dj % 2) * SQ
            return t[:, g, base + c0:base + c1]

        def load_woq(q4):
            nc.gpsimd.dma_start(
                out=wo_ab[q4 // 2][:, :, (q4 % 2) * SQ:(q4 % 2) * SQ + SQ],
                in_=wo[:, q4 * SQ:(q4 + 1) * SQ].rearrange(
                    "(k p) m -> p k m", p=128))

        # ---------- attention section ----------
        pending = [None]

        def norm1(po, acc, si, aob, h, b):
            CUR[0] = f'norm1:{h}{b}:si{si}'
            ps = pscp.tile([1, SQ], dt.float32, name="ps", tag="psc")
            nc.tensor.matmul(ps[:], lhsT=ob_sb[:], rhs=acc[:],
                             start=True, stop=True)
            rec = recp.tile([1, SQ], dt.float32, name="rec", tag="rec")
            nc.vector.reciprocal(out=rec[:], in_=ps[:])
            return rec

        def norm2(po, acc, si, aob, h, b, rec):
            CUR[0] = f'norm2:{h}{b}:si{si}'
            pbc = pscp.tile([128, SQ], dt.float32, name="pbc", tag="psc")
            nc.tensor.matmul(pbc[:], lhsT=of_sb[:], rhs=rec[:],
                             start=True, stop=True)
            # HW reads at most one non-scalar PSUM input per DVE op
            bc = bcp.tile([128, SQ], dt.bfloat16, name="bc", tag="bc")
            nc.vector.tensor_copy(out=bc[:], in_=pbc[:])
            nc.vector.tensor_mul(out=aob[:, si * SQ:(si + 1) * SQ],
                                 in0=po[:], in1=bc[:])
            ship_chunk(h, b, aob, si)
            if si == NB - 1:
                fire_a2a(h, b)

        def fire_a2a(h, b):
            if not sim_mode:
                nc.gpsimd.collective_compute(
                    "AllToAll", mybir.AluOpType.bypass,
                    replica_groups=[list(range(N_CORES))],
                    ins=[ao_in[h][b][:]], outs=[ao_ex[h][b][:]])
            # sim mode: ao_ex is aliased to ao_in, nothing to emit

        def ship_chunk(h, b, aob, si):
            # chunk si of ao_in is exactly [HD, 512]: no rearrange needed.
            # gpsimd queue: SP is issue-rate-limited in the steady state.
            nc.gpsimd.dma_start(out=ao_in[h][b][si],
                                in_=aob[:, si * SQ:(si + 1) * SQ])

        def section(h, b):
            qt, kt, vn = qts[b], kts[b], vns[b]
            aob = aobp.tile([HD, S], dt.bfloat16, name="aob", tag="aob")
            pair_ix = 0
            for si in range(NB):
                CUR[0] = f'sec{h}{b}:si{si}'
                nkv = (si + 1) * DIAG
                po = pop.tile([HD, SQ], dt.float32, name="po", tag="po")
                acc = accp.tile([128, SQ], dt.bfloat16, name="acc", tag="acc")
                for j2 in range(nkv // 2):
                    psc = pscp.tile([128, 2 * SQ], dt.float32, name="psc",
                                    tag="psc")
                    et = etp.tile([128, 2 * SQ], dt.bfloat16, name="et",
                                  tag="et")
                    for jj in range(2):
                        j = j2 * 2 + jj
                        nc.tensor.matmul(
                            psc[:, jj * SQ:(jj + 1) * SQ],
                            lhsT=kt[:, j * KV:(j + 1) * KV],
                            rhs=qt[:, h, si * SQ:(si + 1) * SQ],
                            start=True, stop=True)
                    nc.scalar.activation(et[:], psc[:],
                                         mybir.ActivationFunctionType.Exp,
                                         scale=SCALE)
                    for jj in range(2):
                        dd = j2 * 2 + jj - si * DIAG
                        if dd >= 0:       # causal diagonal block
                            nc.vector.tensor_mul(
                                out=et[:, jj * SQ:(jj + 1) * SQ],
                                in0=et[:, jj * SQ:(jj + 1) * SQ],
                                in1=mask_sb[:, dd * SQ:(dd + 1) * SQ])
                    if pair_ix >= 2:
                        filler.take()
                    pair_ix += 1
                    for jj in range(2):
                        j = j2 * 2 + jj
                        # diagonal blocks: columns below the block's first
                        # unmasked query are all-zero in et; skip them
                        off = max(0, (j - si * DIAG)) * KV
                        nc.tensor.matmul(
                            po[:, off:SQ], lhsT=vn[:, j, :],
                            rhs=et[:, jj * SQ + off:(jj + 1) * SQ],
                            start=(j == 0), stop=(j == nkv - 1))
                    # acc adds and the previous q-block's normalize go AFTER
                    # the avs: they gate only later norm stages, and keeping
                    # them off the exp->mask->av DVE path removes PE stalls
                    if j2 == 0:
                        nc.vector.tensor_add(out=acc[:], in0=et[:, 0:SQ],
                                             in1=et[:, SQ:2 * SQ])
                    else:
                        nc.vector.tensor_add(out=acc[:], in0=acc[:],
                                             in1=et[:, 0:SQ])
                        nc.vector.tensor_add(out=acc[:], in0=acc[:],
                                             in1=et[:, SQ:2 * SQ])
                    if j2 == 0 and pending[0] is not None:
                        nargs = pending[0]
                        rec = norm1(*nargs)
                        pending[0] = (*nargs, rec)
                    elif j2 == 1 and pending[0] is not None:
                        *nargs, rec = pending[0]
                        norm2(*nargs, rec)
                        pending[0] = None
                pending[0] = (po, acc, si, aob, h, b)
            # flush the last q-block eagerly: the section's AllToAll must be
            # emitted before the next section's filler touches its output
            flush_pending()

        def flush_pending():
            nargs = pending[0]
            rec = norm1(*nargs)
            norm2(*nargs, rec)
            pending[0] = None

        # ---------- projections (+RoPE, k/v pair exchange) ----------
        with tc.tile_pool(name="projw", bufs=1) as wp, \
             tc.tile_pool(name="tabs", bufs=1) as tabp, \
             tc.tile_pool(name="qkv0", bufs=1) as q0p, \
             tc.tile_pool(name="pax", bufs=2) as pax, \
             tc.tile_pool(name="ropec", bufs=3) as rcp, \
             tc.tile_pool(name="ropet", bufs=2) as rtp, \
             tc.tile_pool(name="pa2", bufs=2, space="PSUM") as pa2:
            wq_sb = wp.tile([128, DCH, HPC * HD], dt.bfloat16, name="wq_sb")
            wkv_sb = wp.tile([128, DCH, HD], dt.bfloat16, name="wkv_sb")
            cos_sb = tabp.tile([HD, S], dt.bfloat16, name="cos_sb")
            sin_sb = tabp.tile([HD, S], dt.bfloat16, name="sin_sb")
            kvf_sb = tabp.tile([128, 1], dt.float32, name="kvf_sb")
            kvi_sb = tabp.tile([128, 1], dt.float32, name="kvi_sb")

            qts[0] = q0p.tile([HD, HPC, S], dt.bfloat16, name="qt0")
            kts[0] = q0p.tile([HD, S], dt.bfloat16, name="kt0")
            vns[0] = q0p.tile([128, NKV_B, HD], dt.bfloat16, name="vn0")
            ktvs[0] = ktvp.tile([HD, S], dt.bfloat16, name="ktv0", tag="ktv")

            xs_pre = {}

            def load_xs(b, si, split=False):
                xs = pax.tile([128, DCH, SQ], dt.bfloat16, name="xs", tag="xs")
                xr = xT[:, b * S + si * SQ:b * S + si * SQ + SQ].rearrange(
                    "(k p) n -> p k n", p=128)
                if split:
                    hh = DCH // 2
                    nc.sync.dma_start(out=xs[:, 0:hh, :], in_=xr[:, 0:hh, :])
                    nc.sync.dma_start(out=xs[:, hh:DCH, :], in_=xr[:, hh:DCH, :])
                else:
                    nc.sync.dma_start(out=xs[:], in_=xr)
                return xs

            # weight chunk 0 first (the k=0 matmul's gate), then the first
            # x-tile, then the rest — the modeled DMA channel is serial-ish
            wqr = wq[:].rearrange("(k p) m -> p k m", p=128)
            wkr = wkv[:].rearrange("(k p) m -> p k m", p=128)
            nc.sync.dma_start(out=kvf_sb[:], in_=kvf[:])
            nc.sync.dma_start(out=kvi_sb[:], in_=kvi[:])
            nc.sync.dma_start(out=wq_sb[:, 0:2, :], in_=wqr[:, 0:2, :])
            xs_pre[(0, 0)] = load_xs(0, 0, split=True)
            nc.sync.dma_start(out=wq_sb[:, 2:8, :], in_=wqr[:, 2:8, :])
            nc.sync.dma_start(out=wkv_sb[:, 0:8, :], in_=wkr[:, 0:8, :])
            nc.sync.dma_start(out=wq_sb[:, 8:DCH, :], in_=wqr[:, 8:DCH, :])
            nc.sync.dma_start(out=wkv_sb[:, 8:DCH, :], in_=wkr[:, 8:DCH, :])

            def exchange_kv(b, hx):
                # ship this core's k-or-v half, swap with the partner, then
                # (deferred, so the network latency is off the DMA queues)
                # pull the exchanged K half and DMA-transpose the V half.
                c0, c1 = hx * (S // 2), (hx + 1) * (S // 2)
                nc.sync.dma_start(out=ktv_dram[b][hx], in_=ktvs[b][:, c0:c1])
                if not sim_mode:
                    nc.gpsimd.collective_compute(
                        "AllGather", mybir.AluOpType.bypass,
                        replica_groups=[[2 * g, 2 * g + 1]
                                        for g in range(N_CORES // 2)],
                        ins=[ktv_dram[b][hx]], outs=[kv_pair[b][hx]])
                # sim mode: pulls below read ktv_dram directly (aliased)
                kvp_k = kv_pair[b][hx, 0] if not sim_mode else ktv_dram[b][hx]
                kvp_v = kv_pair[b][hx, 1] if not sim_mode else ktv_dram[b][hx]

                def pull_k():
                    CUR[0] = f'pullk{b}:{hx}'
                    nc.sync.dma_start(out=kts[b][:, c0:c1], in_=kvp_k)

                def pull_v():
                    CUR[0] = f'pullv{b}:{hx}'
                    # one batched xbar transpose for the whole half
                    eng = nc.scalar if b == 0 else nc.sync
                    eng.dma_start_transpose(
                        out=vns[b][:, hx * 16:hx * 16 + 16, :], in_=kvp_v[:])

                # deferred past the exchange latency so the queue never
                # stalls on the collective semaphore
                filler.defer(8, pull_k)
                filler.defer(12, pull_v)

            def rope_unit(pqk, unit, qt, ktv_sb, l0):
                # RoPE: copy to bf16 on the scalar engine, then 2x-rate DVE
                # muls; dest = p*cos + rot(p)*sin_signed. rotate-half uses a
                # shifted single-input copy (the SB+SB same-base rule
                # NCC_IBIR297 only restricts two-input TensorTensor ops).
                cb = rcp.tile([128, SQ], dt.bfloat16, name="cb", tag="cb")
                nc.scalar.activation(
                    cb[:], pqk[:], mybir.ActivationFunctionType.Copy)
                dest = (qt[:, unit, l0:l0 + SQ] if unit < 2
                        else ktv_sb[:, l0:l0 + SQ])
                t1 = rtp.tile([128, SQ], dt.bfloat16, name="t1", tag="t1")
                t2 = rtp.tile([128, SQ], dt.bfloat16, name="t2", tag="t2")
                cr = rtp.tile([128, SQ], dt.bfloat16, name="cr", tag="cr")
                nc.vector.tensor_mul(out=t1[:], in0=cb[:],
                                     in1=cos_sb[:, l0:l0 + SQ])
                nc.vector.tensor_copy(out=cr[0:64, :], in_=cb[64:128, :])
                nc.vector.tensor_copy(out=cr[64:128, :], in_=cb[0:64, :])
                nc.vector.tensor_mul(out=t2[:], in0=cr[:],
                                     in1=sin_sb[:, l0:l0 + SQ])
                if unit < 2:
                    nc.vector.tensor_add(out=dest, in0=t1[:], in1=t2[:])
                else:
                    # dest = rope*flag + cb*(1-flag): K cores get rope'd k,
                    # V cores pass v through untouched
                    nc.vector.tensor_add(out=t2[:], in0=t1[:], in1=t2[:])
                    nc.vector.tensor_scalar_mul(out=t2[:], in0=t2[:],
                                                scalar1=kvf_sb[:])
                    nc.vector.tensor_scalar_mul(out=t1[:], in0=cb[:],
                                                scalar1=kvi_sb[:])
                    nc.vector.tensor_add(out=dest, in0=t1[:], in1=t2[:])

            def proj_units(b):
                qt, ktv_sb = qts[b], ktvs[b]
                for si in range(NB):
                    CUR[0] = f'proj{b}:si{si}'
                    xs = xs_pre.pop((b, si), None)
                    if xs is None:
                        xs = load_xs(b, si)
                    if b == 0 and si in (0, 1, 2, 3):
                        # rope table quarter q lands just behind x-tile q:
                        # rope work tolerates ~1 unit of table lag
                        th0, th1 = si * (S // 4), (si + 1) * (S // 4)
                        for tsb, tdr in ((cos_sb, cosT), (sin_sb, sinTs)):
                            nc.sync.dma_start(out=tsb[:, th0:th1],
                                              in_=tdr[:, th0:th1])
                    # prefetch one si ahead: the pool slot's WAR wait ends
                    # roughly now, giving the load a full si of lead time
                    nxt = (b, si + 1) if si + 1 < NB else (
                        (1, 0) if b == 0 else None)
                    if b == 0 and si == NB - 1:
                        nxt = (1, 1)
                    if nxt is not None and nxt not in xs_pre and nxt[1] < NB:
                        xs_pre[nxt] = load_xs(*nxt)
                    if b == 0 and si == 1:
                        nc.sync.dma_start(out=mask_sb[:], in_=masks[:])
                        nc.sync.dma_start(out=ob_sb[:], in_=onesb[:])
                        nc.sync.dma_start(out=of_sb[:], in_=onesf[:])
                    l0 = si * SQ
                    for unit in range(3):
                        pqk = pa2.tile([128, SQ], dt.float32, name="pqk",
                                       tag="pqk")
                        for k2 in range(DCH // 2):
                            for k in (2 * k2, 2 * k2 + 1):
                                lhsT = (wq_sb[:, k, unit * 128:(unit + 1) * 128]
                                        if unit < 2 else wkv_sb[:, k, :])
                                nc.tensor.matmul(pqk[:], lhsT=lhsT,
                                                 rhs=xs[:, k, :],
                                                 start=(k == 0),
                                                 stop=(k == DCH - 1))
                            yield
                        rope_unit(pqk, unit, qt, ktv_sb, l0)
                        yield
                    if si in (NB // 2 - 1, NB - 1):
                        exchange_kv(b, si // (NB // 2))
                        yield

            # ---------- head: batch-0 projections, then S1/S2 ----------
            filler.set_gen(proj_units(0), 0.0)
            filler.drain(thunks=False)
            filler.set_gen(proj_units(1), 1.7)
            section(0, 0)
            load_woq(0)
            load_woq(1)
            section(1, 0)
            filler.drain(thunks=False)

        # ---------- S3/S4 with o_proj filler, then tail ----------
        with tc.tile_pool(name="wopb", bufs=1) as wopb, \
             tc.tile_pool(name="ltp", bufs=2) as ltp, \
             tc.tile_pool(name="stashp", bufs=16) as stp, \
             tc.tile_pool(name="otp", bufs=3) as otp, \
             tc.tile_pool(name="podp", bufs=2, space="PSUM") as podp:
            wo_ab[1] = wopb.tile([128, DCH, D // 2], dt.bfloat16,
                                 name="wo_b")
            for q4 in range(2, 4):
                load_woq(q4)

            stash = {}

            def d_units(rows, hps):
                for r in rows:
                    CUR[0] = f'd:r{r}'
                    bb = 0 if r < 4 else 1
                    lts = {}
                    for hp in hps:
                        lt = ltp.tile([128, N_CORES, 128], dt.bfloat16,
                                      name="lt", tag=f"lt{hp}")
                        nc.gpsimd.dma_start(
                            out=lt[:],
                            in_=ao_ex[hp][bb][:, :, (r % 4) * 128:
                                              (r % 4) * 128 + 128].rearrange(
                                "a p n -> p a n"))
                        lts[hp] = lt
                    for dj in range(D // SQ):
                        pod = podp.tile([128, SQ], dt.float32, name="pod",
                                        tag="pod")
                        for hp in hps:
                            for j in range(N_CORES):
                                g = 2 * j + hp
                                nc.tensor.matmul(
                                    pod[:], lhsT=lts[hp][:, j, :],
                                    rhs=wo_slice(g, dj, 0, SQ),
                                    start=(hp == hps[0] and j == 0),
                                    stop=(hp == hps[-1] and j == N_CORES - 1))
                                if j % 4 == 3:
                                    yield
                        if len(hps) == 2:
                            ot = otp.tile([128, SQ], dt.bfloat16, name="ot",
                                          tag="ot")
                            nc.scalar.activation(
                                ot[:], pod[:],
                                mybir.ActivationFunctionType.Copy)
                            nc.gpsimd.dma_start(
                                out=out[r * 128:(r + 1) * 128,
                                        dj * SQ:(dj + 1) * SQ],
                                in_=ot[:])
                        else:
                            stt = stp.tile([128, SQ], dt.bfloat16, name="stt",
                                           tag="stt")
                            nc.scalar.activation(
                                stt[:], pod[:],
                                mybir.ActivationFunctionType.Copy)
                            stash[(r, dj)] = stt
                        yield

            def chain(*gens):
                for g in gens:
                    yield from g

            filler.set_gen(d_units([0, 1, 2], [0, 1]), 0.9)
            for _ in range(3):
                filler._one()
            section(0, 1)
            filler.drain()
            filler.set_gen(chain(d_units([3], [0, 1]), d_units([4, 5, 6, 7], [0])),
                           1.0)
            for _ in range(3):
                filler._one()
            section(1, 1)
            filler.drain()

            # tail: odd heads of batch-1 rows + combine with stashed partials
            for r in (4, 5, 6, 7):
                CUR[0] = f'tail:r{r}'
                lt = ltp.tile([128, N_CORES, 128], dt.bfloat16,
                              name="lt", tag="lt1")
                # chunks 0-6 shipped well before the final normalize: load
                # them separately so the j<7 matmuls aren't gated on chunk 7
                nc.gpsimd.dma_start(
                    out=lt[:, 0:7, :],
                    in_=ao_ex[1][1][0:7, :, (r % 4) * 128:
                                    (r % 4) * 128 + 128].rearrange(
                        "a p n -> p a n"))
                nc.gpsimd.dma_start(
                    out=lt[:, 7:8, :],
                    in_=ao_ex[1][1][7:8, :, (r % 4) * 128:
                                    (r % 4) * 128 + 128].rearrange(
                        "a p n -> p a n"))
                for dj in range(D // SQ):
                    pod = podp.tile([128, SQ], dt.float32, name="pod",
                                    tag="pod")
                    for j in range(N_CORES):
                        nc.tensor.matmul(
                            pod[:], lhsT=lt[:, j, :],
                            rhs=wo_slice(2 * j + 1, dj, 0, SQ),
                            start=(j == 0), stop=(j == N_CORES - 1))
                    ot = otp.tile([128, SQ], dt.bfloat16, name="ot", tag="ot")
                    nc.vector.tensor_add(out=ot[:], in0=pod[:],
                                         in1=stash[(r, dj)][:])
                    nc.gpsimd.dma_start(
                        out=out[r * 128:(r + 1) * 128, dj * SQ:(dj + 1) * SQ],
                        in_=ot[:])

    nc.compile()
    return nc


def _host_prep(x, cos, sin, wq, wk, wv, wo):
    x = np.asarray(x, dtype=np.float32)
    cos = np.asarray(cos, dtype=np.float32)
    sin = np.asarray(sin, dtype=np.float32)
    wq = np.asarray(wq, dtype=np.float32)
    wk = np.asarray(wk, dtype=np.float32)
    wv = np.asarray(wv, dtype=np.float32)
    wo = np.asarray(wo, dtype=np.float32)

    xT = np.ascontiguousarray(x.reshape(BS, D).T.astype(BF16))         # [D, BS]
    cosT = np.ascontiguousarray(cos[0].T)                              # [HD, S]
    sinT = np.ascontiguousarray(sin[0].T).copy()
    sinT[:64] = -sinT[:64]                      # fold rotate_half sign into sin

    # causal diagonal masks: mask[d][r, c] = 1 iff query col c >= key (d*128+r)
    cc = np.arange(SQ)[None, :]
    rr = np.arange(128)[:, None]
    mtiles = [(cc >= d * 128 + rr).astype(np.float32) for d in range(DIAG)]
    masks = np.ascontiguousarray(np.concatenate(mtiles, axis=1).astype(BF16))

    onesb = np.ones((128, 1), dtype=np.float32).astype(BF16)
    onesf = np.ones((1, 128), dtype=np.float32)

    wq_bf = wq.astype(BF16)
    wk_bf = wk.astype(BF16)
    wv_bf = wv.astype(BF16)
    wo_bf = np.ascontiguousarray(wo.astype(BF16))

    cos_bf = cosT.astype(BF16)
    sin_bf = sinT.astype(BF16)
    ones_col = np.ones((128, 1), dtype=np.float32)
    zeros_col = np.zeros((128, 1), dtype=np.float32)

    in_maps = []
    for c in range(N_CORES):
        kvh = c // 2
        is_k_core = (c % 2 == 0)
        wkv_full = wk_bf if is_k_core else wv_bf
        in_maps.append({
            "xT": xT,
            "cosT": cos_bf,
            "sinTs": sin_bf,
            "kvf": ones_col if is_k_core else zeros_col,
            "kvi": zeros_col if is_k_core else ones_col,
            "wq": np.ascontiguousarray(wq_bf[:, c * HPC * HD:(c + 1) * HPC * HD]),
            "wkv": np.ascontiguousarray(wkv_full[:, kvh * HD:(kvh + 1) * HD]),
            "wo": wo_bf,
            "masks": masks,
            "onesb": onesb,
            "onesf": onesf,
        })
    return in_maps


def kernel(x, cos, sin, wq, wk, wv, wo):
    from concourse.bass_utils import run_bass_kernel_spmd

    if "nc" not in _CACHE:
        _CACHE["nc"] = _build()
    nc = _CACHE["nc"]

    in_maps = _host_prep(x, cos, sin, wq, wk, wv, wo)
    res = run_bass_kernel_spmd(nc, in_maps, core_ids=list(range(N_CORES)))
    full = np.empty((BS, D), dtype=np.float32)
    for c in range(N_CORES):
        o = res.results[c]["out"]
        full[c * HSH:(c + 1) * HSH] = o[0:HSH]
        full[S + c * HSH:S + (c + 1) * HSH] = o[HSH:SHARD]
    return full.reshape(B, S, D)
